# revision 44
# baseline (speedup 1.0000x reference)
"""Distributed multi-head attention kernel for one TRN2 chip (8 NeuronCores).

Problem: x[4, 2048, 1024] -> qkv Linear(1024, 3072, bias=False) -> 16-head
softmax attention -> proj Linear(1024, 1024) + bias.

Sharding: tensor-parallel over heads. Core c owns heads {2c, 2c+1} (128 of the
1024 qkv feature dims). Each core computes Q/K/V for its head pair over the
full sequence and runs attention per (batch, head). For batches 0-2 the cores
reshard with one AllToAll per half batch (1024 tokens) so core c ends up with
the full 1024 attention features for a 128-token slice of each half, then
applies the full W_proj to its token slices. For batch 3 (the tail) the proj
is row-parallel instead: each core multiplies its own 128 attention features
by its slice of W_proj rows and emits a PARTIAL [2048, 1024] output; the host
sums the 8 partials. This removes the last two collectives from the critical
path entirely (a collective costs ~20us wall regardless of size).

The proj bias is added on the host, so no collective-dependent op ever lands
on the Vector queue ahead of the softmax-drain chain (in-order queues would
otherwise cascade every AllToAll ~40us late, as seen in the v1 trace).

Engine schedule: the interleave pairs step k of chunk c's score matmuls with
step k-OFF of chunk c-1's PV matmuls; QK/V/proj matmuls are woven in as
ACT-independent ballast at fixed slots so the PE never idles while the exp
stream paces the chunk. Every proj consumption trails its AllToAll trigger by
>=1.5 chunks (collectives here take 10-55us depending on environment load),
and nothing that waits on a collective ever precedes collective-independent
work on the same in-order engine queue -- that ordering, not bandwidth, was
the dominant failure mode in earlier versions.

Layout notes:
 - x is transposed on the host to xT [C, B*N] so SBUF tiles have the
   contraction dim (C) on partitions for the QK matmuls.
 - Q and K are produced transposed (QT/KT [128 head-dims, tokens]) which is
   exactly the operand layout for S^T = K Q^T. Scores are built transposed
   (ST [k_tok, q_tok]) so that P^T is directly the lhs-side operand of the
   PV matmul (k_tok on partitions).
 - V is computed directly in natural [token, head-dim] layout (x-tile as the
   stationary operand) with an extra all-ones column per head, so the PV
   matmul also yields the softmax denominator row for free. No PE transposes.
 - No row-max subtraction: scores are ~N(0,1) after scaling so exp is safe.
"""

import os
import sys

import numpy as np

for _p in ("/opt/trn_rl_repo", "/root/.axon_site/_ro/trn_rl_repo"):
    if os.path.isdir(_p) and _p not in sys.path:
        sys.path.append(_p)

import ml_dtypes  # noqa: E402

B, N, C = 4, 2048, 1024
NUM_HEADS = 16
HEAD_DIM = C // NUM_HEADS  # 64
SCALE = HEAD_DIM**-0.5
NCORES = 8
P = 128  # SBUF partitions
QC = 512  # q-chunk (matmul free dim / PSUM bank)
TPH = 128  # tokens per core per half-batch after reshard
NA2A = 6  # halves resharded via AllToAll (batches 0-2); batch 3 row-parallel

BF16 = ml_dtypes.bfloat16


def build_attention_nc(NB: int = B, NQ: int = N, CH: int = C):
    """Build + compile the SPMD graph. NB batches of NQ tokens, CH channels."""
    import concourse.bass as bass
    import concourse.mybir as mybir
    import concourse.tile as tile
    from concourse import bacc

    f32 = mybir.dt.float32
    bf16 = mybir.dt.bfloat16

    n_qc = NQ // QC          # q chunks per batch (4)
    n_kt = NQ // P           # k tiles per batch (16)
    n_cc = CH // P           # contraction chunks (8)
    n_ck = NB * n_qc         # total chunks (16)

    nc = bacc.Bacc("TRN2", target_bir_lowering=False, debug=False,
                   num_devices=NCORES)

    xT = nc.dram_tensor("xT", [CH, NB * NQ], bf16, kind="ExternalInput").ap()
    wq = nc.dram_tensor("wq", [CH, P], bf16, kind="ExternalInput").ap()
    wk = nc.dram_tensor("wk", [CH, P], bf16, kind="ExternalInput").ap()
    wv = nc.dram_tensor("wv", [CH, P], bf16, kind="ExternalInput").ap()
    wp = nc.dram_tensor("wp", [CH, CH], bf16, kind="ExternalInput").ap()
    wpo = nc.dram_tensor("wpo", [P, CH], bf16, kind="ExternalInput").ap()
    out = nc.dram_tensor("out", [NA2A * TPH, CH], f32,
                         kind="ExternalOutput").ap()
    out2 = nc.dram_tensor("out2", [NQ, CH], bf16,
                          kind="ExternalOutput").ap()

    from contextlib import ExitStack

    with tile.TileContext(nc) as tc, ExitStack() as ctx:
        const = ctx.enter_context(tc.tile_pool(name="const", bufs=1))
        xt_pool = ctx.enter_context(tc.tile_pool(name="xt", bufs=9))
        qk_pool = ctx.enter_context(tc.tile_pool(name="qk", bufs=2))
        v_pool = ctx.enter_context(tc.tile_pool(name="v", bufs=2))
        pt_pool = ctx.enter_context(tc.tile_pool(name="pt", bufs=2))
        ot_pool = ctx.enter_context(tc.tile_pool(name="ot", bufs=2))
        num_pool = ctx.enter_context(tc.tile_pool(name="num", bufs=2))
        div_pool = ctx.enter_context(tc.tile_pool(name="div", bufs=2))
        at_pool = ctx.enter_context(tc.tile_pool(name="at", bufs=3))
        y_pool = ctx.enter_context(tc.tile_pool(name="y", bufs=3))
        py_pool = ctx.enter_context(tc.tile_pool(name="py", bufs=6))
        dram = ctx.enter_context(tc.tile_pool(name="dram", bufs=1, space="DRAM"))
        ps_st = ctx.enter_context(tc.tile_pool(name="ps_st", bufs=2, space="PSUM"))
        ps_pv = ctx.enter_context(tc.tile_pool(name="ps_pv", bufs=1, space="PSUM"))
        ps_mm = ctx.enter_context(tc.tile_pool(name="ps_mm", bufs=2, space="PSUM"))

        # Pre-warm the collective engine first thing on the gpsimd queue: the
        # first real AllToAll otherwise pays the ~70us one-time CC init.
        cc_warm_in = dram.tile([NCORES, 32], bf16, tag="ccw_i", name="ccw_i")
        cc_warm_out = dram.tile([NCORES, 32], bf16, tag="ccw_o", name="ccw_o")
        nc.gpsimd.collective_compute(
            "AllToAll", mybir.AluOpType.bypass,
            replica_groups=[list(range(NCORES))],
            ins=[cc_warm_in[:].opt()], outs=[cc_warm_out[:].opt()])

        # --- resident weights ---
        # wq/wk go on the sync queue ahead of the x^T tiles (the first QK
        # chains need them immediately); wv/wp/wpo can trail.
        wq_sb = const.tile([P, n_cc, P], bf16, tag="wq")
        wk_sb = const.tile([P, n_cc, P], bf16, tag="wk")
        wv_sb = const.tile([P, n_cc, P], bf16, tag="wv")
        wp_sb = const.tile([P, n_cc, CH], bf16, tag="wp")
        wpo_sb = const.tile([P, CH], bf16, tag="wpo")
        nc.sync.dma_start(wq_sb[:], wq.rearrange("(cc p) m -> p cc m", p=P))
        nc.scalar.dma_start(wk_sb[:], wk.rearrange("(cc p) m -> p cc m", p=P))
        nc.gpsimd.dma_start(wv_sb[:], wv.rearrange("(cc p) m -> p cc m", p=P))

        # Pre-warm the ACT exp table during the initial DMA window so the
        # first real exp doesn't eat the ~2.7us table load.
        warm = const.tile([1, 2], f32, tag="warm")
        nc.vector.memset(warm[:, 0:1], 0.0)
        nc.scalar.activation(warm[:, 1:2], warm[:, 0:1],
                             mybir.ActivationFunctionType.Exp)

        a2a_in = [None] * NB
        a2a_out = [None] * NB
        for b in range(NB - 1):
            a2a_in[b] = dram.tile([NCORES * P, 2 * TPH], bf16,
                                  tag=f"a2a_in{b}", name=f"a2a_in{b}")
            a2a_out[b] = dram.tile([NCORES * P, 2 * TPH], bf16,
                                   tag=f"a2a_out{b}", name=f"a2a_out{b}")

        # per-batch state
        xts = {}      # b -> list of n_cc xt tiles
        qts, kts = {}, {}
        v_sbs = {}
        ots = {}
        pts = {}      # chunk c -> pt tile
        pvps = {}     # chunk c -> pv psum tile

        def emit_xt(b, split=False):
            # With 9 pool bufs the SECOND allocated tile of each batch lands
            # on a buffer still read by the previous batch's chains, so its
            # DMA is held back ~a chunk. Allocate cc7 (the last-consumed
            # contraction chunk) in that position so nothing ever waits.
            order = [0, 7, 1, 2, 3, 4, 5, 6]
            tiles = [None] * n_cc
            for cc in order:
                tiles[cc] = xt_pool.tile([P, NQ], bf16, tag="xt",
                                         name="xt_tile")
            if split:
                # half-token loads ordered to match the prologue QK chains:
                # u1 of the first pair needs cc 0-3 of the first token half,
                # u2 needs cc 4-7, then the second half. 16 dispatches total.
                hw = NQ // 2
                engs = [nc.sync, nc.scalar]
                i = 0
                for hp in range(2):
                    for cg in range(2):
                        for cc in range(4 * cg, 4 * cg + 4):
                            # round-robin the issuing DGE queues (SP + Activation): descriptor
                            # generation is the startup bottleneck, and three
                            # queues generate in parallel
                            engs[i % 2].dma_start(
                                tiles[cc][:, hp * hw:(hp + 1) * hw],
                                xT[cc * P:(cc + 1) * P,
                                   b * NQ + hp * hw:b * NQ + (hp + 1) * hw])
                            i += 1
            else:
                for cc in order:
                    nc.sync.dma_start(
                        tiles[cc][:],
                        xT[cc * P:(cc + 1) * P, b * NQ:(b + 1) * NQ])
            xts[b] = tiles

        def qk_units(b):
            """Q^T / K^T matmuls for batch b as 4 (u1, u2) pairs of 8 matmuls.
            A pair shares PSUM tiles; no other tag-mm allocation may be
            emitted between u1 and u2."""
            qt_sb = qk_pool.tile([P, NQ], bf16, tag="qt", name="qt_sb")
            kt_sb = qk_pool.tile([P, NQ], bf16, tag="kt", name="kt_sb")
            qts[b], kts[b] = qt_sb, kt_sb
            pairs = []
            for qc in range(0, n_qc, 2):
                for w_sb, dst in ((wq_sb, qt_sb), (wk_sb, kt_sb)):
                    qsa = slice(qc * QC, (qc + 1) * QC)
                    qsb = slice((qc + 1) * QC, (qc + 2) * QC)
                    st = {}

                    def u1(w_sb=w_sb, qsa=qsa, qsb=qsb, st=st):
                        st["psa"] = ps_mm.tile([P, QC], f32, tag="mm",
                                               name="ps_a")
                        st["psb"] = ps_mm.tile([P, QC], f32, tag="mm",
                                               name="ps_b")
                        for cc in range(n_cc // 2):
                            nc.tensor.matmul(st["psa"][:], w_sb[:, cc, :],
                                             xts[b][cc][:, qsa],
                                             start=(cc == 0), stop=False)
                            nc.tensor.matmul(st["psb"][:], w_sb[:, cc, :],
                                             xts[b][cc][:, qsb],
                                             start=(cc == 0), stop=False)

                    def u2(w_sb=w_sb, dst=dst, qsa=qsa, qsb=qsb, st=st):
                        for cc in range(n_cc // 2, n_cc):
                            nc.tensor.matmul(st["psa"][:], w_sb[:, cc, :],
                                             xts[b][cc][:, qsa],
                                             start=False,
                                             stop=(cc == n_cc - 1))
                            nc.tensor.matmul(st["psb"][:], w_sb[:, cc, :],
                                             xts[b][cc][:, qsb],
                                             start=False,
                                             stop=(cc == n_cc - 1))
                        nc.vector.tensor_copy(dst[:, qsa], st["psa"][:])
                        nc.vector.tensor_copy(dst[:, qsb], st["psb"][:])

                    pairs.append((u1, u2))
            return pairs

        def alloc_v(b):
            # V natural layout + ones column per head: [tok-tile, 2x(64+1)]
            v_sb = v_pool.tile([P, n_kt, 130], bf16, tag="v", name="v_sb")
            ones_view = v_sb.rearrange("p t (g c) -> p t g c", g=2)[:, :, :, 64:65]
            nc.vector.memset(ones_view, 1.0)
            v_sbs[b] = v_sb

        def v_units(b):
            """V for batch b as 4 slot units of 4 token tiles each, scheduled
            in batch b-1's qc2/qc3 chunks (xt(b) is resident there). This
            ends xt(b)'s lifetime a chunk earlier, so batch b+1's x loads
            are not held back by pool-buffer eviction waits."""
            def make(tt0, first):
                def u():
                    if first:
                        alloc_v(b)
                    for tt in range(tt0, tt0 + 4):
                        emit_v_tile(b, tt)
                return u
            return [make(tt0, tt0 == 0) for tt0 in range(0, n_kt, 4)]

        def emit_v_tile(b, tt):
            """One 128-token tile of V, directly in natural layout: x-tile is
            the stationary operand, accumulate over contraction chunks."""
            vps = ps_mm.tile([P, P], f32, tag="mm", name="vps")
            ts_ = slice(tt * P, (tt + 1) * P)
            for cc in range(n_cc):
                nc.tensor.matmul(vps[:], xts[b][cc][:, ts_], wv_sb[:, cc, :],
                                 start=(cc == 0), stop=(cc == n_cc - 1))
            dst = v_sbs[b].rearrange("p t (g c) -> p t g c", g=2)[:, tt, :, 0:64]
            nc.vector.tensor_copy(dst, vps.rearrange("p (g c) -> p g c", g=2))

        def emit_scores_step(c, kt):
            """Scores + exp for chunk c, k-tile kt."""
            b, qc = divmod(c, n_qc)
            qs = slice(qc * QC, (qc + 1) * QC)
            ks = slice(kt * P, (kt + 1) * P)
            if kt == 0:
                pts[c] = pt_pool.tile([P, n_kt, 2, QC], bf16, tag="pt",
                                      name="pt_tile")
            st = ps_st.tile([P, 2, QC], f32, tag="st", name="st_tile")
            for h in range(2):
                hs = slice(64 * h, 64 * (h + 1))
                nc.tensor.matmul(st[:, h, :], kts[b][hs, ks], qts[b][hs, qs])
            nc.scalar.activation(pts[c][:, kt, :, :], st[:],
                                 mybir.ActivationFunctionType.Exp, scale=SCALE)

        def emit_pv_step(c, kt):
            b, qc = divmod(c, n_qc)
            if kt == 0:
                pvps[c] = ps_pv.tile([P, 2, QC], f32, tag="pv", name="pv_ps")
            for h in range(2):
                nc.tensor.matmul(
                    pvps[c][0:65, h, :], v_sbs[b][:, kt, 65 * h:65 * (h + 1)],
                    pts[c][:, kt, h, :],
                    start=(kt == 0), stop=(kt == n_kt - 1))

        def emit_drain(c):
            """Move PV numerator+denominator out of PSUM, divide into ot."""
            b, qc = divmod(c, n_qc)
            qs = slice(qc * QC, (qc + 1) * QC)
            if qc == 0:
                ots[b] = ot_pool.tile([P, NQ], bf16, tag="ot", name="ot_sb")
            onum = num_pool.tile([P, 2, QC], f32, tag="onum", name="onum_t")
            # these copies free the pv psum banks for chunk c+1; they stay on
            # the vector queue so the exp stream (scalar) is never delayed.
            # The last two drains go to scalar instead: no exps remain, and
            # the vector queue is full of partial-proj casts at that point.
            if c >= n_ck - 2:
                nc.scalar.copy(onum[0:65, :, :], pvps[c][0:65, :, :])
            else:
                for h in range(2):
                    nc.vector.tensor_copy(onum[0:65, h, :], pvps[c][0:65, h, :])
            for h in range(2):
                drow = div_pool.tile([1, QC], f32, tag="drow", name="drow_t")
                nc.vector.tensor_copy(drow[:], onum[64:65, h, :])
                braw = div_pool.tile([64, QC], f32, tag="braw", name="braw_t")
                nc.gpsimd.partition_broadcast(braw[:], drow[:])
                rec = div_pool.tile([64, QC], f32, tag="rec", name="rec_t")
                nc.vector.reciprocal_approx_fast(rec[:], braw[:])
                nc.vector.tensor_mul(ots[b][64 * h:64 * (h + 1), qs],
                                     onum[0:64, h, :], rec[:])
            del pvps[c]
            del pts[c]

        def emit_a2a(b):
            """Reshard the whole batch b in ONE AllToAll (dest core j gets
            its 128-token slice of both halves). One collective per batch
            instead of two keeps the CC stream far below its issue rate even
            when the environment degrades to ~60us per collective."""
            for h in range(2):
                nc.gpsimd.dma_start(
                    a2a_in[b].rearrange("(j p) (h t) -> p j h t",
                                        p=P, h=2)[:, :, h, :],
                    ots[b].rearrange("p (hh j t) -> p hh j t",
                                     hh=2, j=NCORES)[:, h])
            nc.gpsimd.collective_compute(
                "AllToAll", mybir.AluOpType.bypass,
                replica_groups=[list(range(NCORES))],
                ins=[a2a_in[b][:].opt()], outs=[a2a_out[b][:].opt()])

        def emit_at_load(b, h, eng=None):
            """All 8 feature tiles of one resharded half in a single DMA.
            `eng` picks the issuing queue: the epilogue load goes on the
            scalar DGE so its collective wait cannot dam the sync queue
            (which recycles the partial-proj output tiles)."""
            at = at_pool.tile([P, n_cc, P], bf16, tag="at", name="at_tile")
            (eng or nc.sync).dma_start(
                at[:], a2a_out[b].rearrange("(cc p) (h t) -> p cc h t",
                                            p=P, h=2)[:, :, h, :])
            return at

        def proj_unit(b, h, at, oc):
            """One output-chunk of W_proj for half h of batch b (8 matmuls,
            self-contained tag-mm usage). Bias is added on the host."""
            def u():
                ocs = slice(oc * QC, (oc + 1) * QC)
                yps = ps_mm.tile([P, QC], f32, tag="mm", name="yps_t")
                for cc in range(n_cc):
                    nc.tensor.matmul(yps[:], at[:, cc, :], wp_sb[:, cc, ocs],
                                     start=(cc == 0), stop=(cc == n_cc - 1))
                y_sb = y_pool.tile([P, QC], f32, tag="y", name="y_tile")
                nc.vector.tensor_copy(y_sb[:], yps[:])
                nc.sync.dma_start(
                    out[(b * 2 + h) * TPH:(b * 2 + h + 1) * TPH, ocs],
                    y_sb[:])
            return u

        def pproj_unit(tt, alt_dma=False, cast_scalar=False):
            """Row-parallel partial proj for one 128-token tile of batch 3:
            partial[tok, :] = ot[ourfeats, tok]^T @ wpo. Host sums cores.
            Partials go out in bf16 (copies 2x faster, half the DMA).
            alt_dma routes half the output DMAs to the gpsimd queue so the
            epilogue's DMA drain is not serialized on one queue."""
            def u():
                ts_ = slice(tt * P, (tt + 1) * P)
                for oc in range(2):
                    ocs = slice(oc * QC, (oc + 1) * QC)
                    yps = ps_mm.tile([P, QC], f32, tag="mm", name="pp_t")
                    nc.tensor.matmul(yps[:], ots[NB - 1][:, ts_],
                                     wpo_sb[:, ocs])
                    y_sb = py_pool.tile([P, QC], bf16, tag="py",
                                        name="py_tile")
                    # in-chunk casts ride the vector queue; epilogue casts go
                    # to the scalar engine, which has no exps left to run
                    if cast_scalar:
                        nc.scalar.copy(y_sb[:], yps[:])
                    else:
                        nc.vector.tensor_copy(y_sb[:], yps[:])
                    eng = nc.gpsimd if (alt_dma and oc == 1) else nc.sync
                    eng.dma_start(out2[ts_, ocs], y_sb[:])
            return u

        # ---------------- schedule ----------------
        # Ballast units (QK pairs, proj output-chunks) are woven into the
        # step loop at fixed slots so the PE never idles while the exp
        # stream paces the chunk, and so collective-dependent proj matmuls
        # enter the in-order PE queue only well after their AllToAll
        # completed. V tiles are inlined per-step in each batch's first chunk
        # (tile kt is ready before pv uses it at step kt+OFF of the next
        # chunk). QK u1/u2 pairs sit at adjacent slots with nothing between
        # them that allocates a tag-mm PSUM tile.
        emit_xt(0, split=True)
        # Only the first token-half's Q/K chains run before chunk 0 — the
        # second half's chains slot INTO chunk 0 (kt 1024-2047 is first
        # consumed at score step 8), so chunk 0's scores and the exp stream
        # start ~14us earlier and overlap the second half of the x DMAs.
        pairs0 = qk_units(0)
        for u1, u2 in pairs0[:2]:
            u1()
            u2()
        if NB > 1:
            emit_xt(1)
        nc.sync.dma_start(wpo_sb[:], wpo[:, :])
        nc.sync.dma_start(wp_sb[:], wp.rearrange("(cc p) m -> p cc m", p=P))

        qk_pend = []        # pending QK pairs for the next batch
        v_pend = []         # pending V units for the next batch
        OFF = 5             # pv steps trail scores by OFF steps
        for c in range(n_ck):
            b, qc = divmod(c, n_qc)
            if qc == 0:
                if b == 0:
                    alloc_v(b)
                if b + 1 < NB:
                    qk_pend = qk_units(b + 1)
                    v_pend = v_units(b + 1)
            # build this chunk's unit slot map. proj units are consumed
            # ~1.5 chunks after their AllToAll trigger so the schedule
            # tolerates collectives as slow as ~30us without stalling the
            # in-order PE queue.
            sl = {}
            if c == 0:
                q23, k23 = pairs0[2], pairs0[3]
                sl[6] = lambda: (k23[0](), k23[1]())
                sl[10] = lambda: (q23[0](), q23[1]())
            if qc == 0 and b == 2:
                # batch 0's second proj: extra-late because the first few
                # AllToAlls can run up to ~55us in bad runs
                at = emit_at_load(0, 1)
                sl[13] = proj_unit(0, 1, at, 0)
                sl[15] = proj_unit(0, 1, at, 1)
            if qc == 0 and b == 3:
                # proj(1,0) parked here: its merged AllToAll triggered at
                # chunk 8's end, giving ~4 chunks of slack
                at = emit_at_load(1, 0)
                sl[2] = proj_unit(1, 0, at, 0)
                sl[5] = proj_unit(1, 0, at, 1)
            if qc == 1:
                if qk_pend:
                    u1, u2 = qk_pend.pop(0)
                    sl[2], sl[5] = u1, u2

            if qc == 2:
                for i in range(2):
                    if qk_pend:
                        u1, u2 = qk_pend.pop(0)
                        sl[2 + 6 * i], sl[5 + 6 * i] = u1, u2
                if v_pend:
                    sl[13] = v_pend.pop(0)
                    sl[14] = v_pend.pop(0)
                if b == 3:
                    # partial proj for batch 3 tokens 0-511 (drain(12) done)
                    sl[3] = pproj_unit(0)
                    sl[6] = pproj_unit(1)
                    sl[9] = pproj_unit(2)
                    sl[12] = pproj_unit(3)
            if qc == 3:
                if qk_pend:
                    u1, u2 = qk_pend.pop(0)
                    sl[2], sl[5] = u1, u2
                if v_pend:
                    sl[12] = v_pend.pop(0)
                    sl[14] = v_pend.pop(0)
                if b == 1:
                    at = emit_at_load(0, 0)
                    sl[13] = proj_unit(0, 0, at, 0)
                    sl[15] = proj_unit(0, 0, at, 1)
                if b == 2:
                    at = emit_at_load(1, 1)
                    sl[8] = proj_unit(1, 1, at, 0)
                    sl[11] = proj_unit(1, 1, at, 1)
                if b == 3:
                    # tokens 512-1023 (drain(13) runs at this chunk's start)
                    sl[3] = pproj_unit(4)
                    sl[6] = pproj_unit(5)
                    sl[9] = pproj_unit(6)
                    sl[12] = pproj_unit(7)

            # interleaved inner loop: pv of the previous chunk runs OFF steps
            # behind scores of this chunk; V tiles of batch b are inlined in
            # the batch's first chunk as ACT-independent ballast.
            for k in range(n_kt + OFF):
                if k >= OFF and c >= 1:
                    emit_pv_step(c - 1, k - OFF)
                if k < n_kt:
                    if c == 0:
                        emit_v_tile(b, k)
                    if k in sl:
                        sl[k]()
                    emit_scores_step(c, k)
            if c >= 1:
                emit_drain(c - 1)
            # boundary work: the merged collective fires as soon as the
            # previous batch's last drain is done
            if qc == 0 and 1 <= b <= NB - 1:
                emit_a2a(b - 1)                         # needs drain(4b-1)
            if qc == 3 and b + 2 < NB:
                emit_xt(b + 2)

        def pproj_st_unit(tt):
            """Final-stretch partial proj using the (now idle) score PSUM
            banks: both output chunks of one token tile land in a single
            [P,2,QC] tile -> one cast -> one DMA. Double-buffered via the
            st pool so the PE never waits on the cast."""
            ts_ = slice(tt * P, (tt + 1) * P)
            yps = ps_st.tile([P, 2, QC], f32, tag="st", name="pst_t")
            for oc in range(2):
                nc.tensor.matmul(yps[:, oc, :], ots[NB - 1][:, ts_],
                                 wpo_sb[:, oc * QC:(oc + 1) * QC])
            for oc in range(2):
                ocs = slice(oc * QC, (oc + 1) * QC)
                y_sb = py_pool.tile([P, QC], bf16, tag="py", name="pst_y")
                # alternate cast engines so the two streams overlap
                if oc == 0:
                    nc.scalar.copy(y_sb[:], yps[:, oc, :])
                    nc.sync.dma_start(out2[ts_, ocs], y_sb[:])
                else:
                    nc.vector.tensor_copy(y_sb[:], yps[:, oc, :])
                    nc.gpsimd.dma_start(out2[ts_, ocs], y_sb[:])

        # epilogue: the last chunk's pv runs interleaved with the remaining
        # partial-proj ballast; no collectives left on the critical path.
        # proj(2,1) and one partial unit fill the drain(15) window (they do
        # not depend on chunk 15's drain).
        last = n_ck - 1
        pp = [pproj_unit(8 + i, alt_dma=True, cast_scalar=True)
              for i in range(4)]  # qc2 tokens
        for k in range(n_kt):
            emit_pv_step(last, k)
            if k in (6, 9, 12):
                pp[(k - 6) // 3]()
        emit_drain(last)
        # at-loads emitted only now, on the gpsimd queue: their collective
        # wait lands after the last divide's broadcast, where nothing that
        # the epilogue pipeline needs can queue up behind it. Both proj(2,*)
        # halves fill the drain(15) window; parking them here (instead of in
        # chunk 15's slots) costs ~3us when collectives are fast but removes
        # a 10-14us stall when the fabric degrades past ~35us per AllToAll.
        at20 = emit_at_load(2, 0, eng=nc.gpsimd)
        at21 = emit_at_load(2, 1, eng=nc.gpsimd)
        proj_unit(2, 0, at20, 0)()
        proj_unit(2, 0, at20, 1)()
        pp[3]()
        proj_unit(2, 1, at21, 0)()
        proj_unit(2, 1, at21, 1)()
        for i in range(4):                          # tokens 1536-2047 (qc3)
            pproj_st_unit(12 + i)

    nc.compile()
    return nc


def make_in_maps(x, W_qkv, W_proj, b_proj, NB=B, NQ=N, CH=C):
    """Shard the full inputs into one input map per core."""
    xT = np.ascontiguousarray(
        x.reshape(NB * NQ, CH).T).astype(BF16)
    wp = np.ascontiguousarray(W_proj).astype(BF16)
    in_maps = []
    for c in range(NCORES):
        cs = slice(P * c, P * (c + 1))
        in_maps.append({
            "xT": xT,
            "wq": np.ascontiguousarray(W_qkv[:, cs]).astype(BF16),
            "wk": np.ascontiguousarray(W_qkv[:, CH:][:, cs]).astype(BF16),
            "wv": np.ascontiguousarray(W_qkv[:, 2 * CH:][:, cs]).astype(BF16),
            "wp": wp,
            "wpo": np.ascontiguousarray(W_proj[cs, :]).astype(BF16),
        })
    return in_maps


def assemble_output(results, b_proj, NB=B, NQ=N, CH=C):
    """Scatter the per-core shards into the full output, sum the batch-3
    row-parallel partials, and add the proj bias."""
    full = np.empty((NB, NQ, CH), dtype=np.float32)
    half = NQ // 2  # 1024
    for c in range(NCORES):
        y = np.asarray(results[c]["out"], dtype=np.float32)
        for b in range(NB - 1):
            for h in range(2):
                dst = half * h + TPH * c
                src = (b * 2 + h) * TPH
                full[b, dst:dst + TPH, :] = y[src:src + TPH]
    acc = np.zeros((NQ, CH), dtype=np.float32)
    for c in range(NCORES):
        acc += np.asarray(results[c]["out2"], dtype=np.float32)
    full[NB - 1] = acc
    full += np.asarray(b_proj, dtype=np.float32)[None, None, :]
    return full


_compiled_nc = None


def kernel(x, W_qkv, W_proj, b_proj):
    global _compiled_nc
    x = np.asarray(x, dtype=np.float32)
    W_qkv = np.asarray(W_qkv, dtype=np.float32)
    W_proj = np.asarray(W_proj, dtype=np.float32)
    b_proj = np.asarray(b_proj, dtype=np.float32)

    if _compiled_nc is None:
        _compiled_nc = build_attention_nc()

    from concourse.bass_utils import run_bass_kernel_spmd

    in_maps = make_in_maps(x, W_qkv, W_proj, b_proj)
    res = run_bass_kernel_spmd(_compiled_nc, in_maps,
                               core_ids=list(range(NCORES)))
    return assemble_output(res.results, b_proj)
